# revision 4
# baseline (speedup 1.0000x reference)
"""Trainium2 kernel for: out = tanh(x @ scatter_nd(nonzero_ind, kernel_vector, (20000, 4096)) + bias).

Strategy v4 (8 NeuronCores, unit-sharded, chunked k-stream):
  - Core c owns the 512-unit column slice W[:, c*512:(c+1)*512]. Both operands
    stream from HBM exactly once per core: x as transposed fp16 [128 x 2048]
    tiles (contiguous 512KB DMAs), W as fp16 [128 x 512] tiles (128KB DMAs),
    spread across both HWDGE rings (sync + scalar) since each dma_start costs
    ~600ns of engine issue time.
  - k-tiles are processed in ramped chunks ([2,3,4,6,8,10] + [12]*10 + [4])
    so each chunk's loads hide under the previous chunks' compute. Early
    chunks' x tiles are split into per-batch-block quarters/halves issued in
    consumption order, so the PE starts on the first 128KB.
  - Within a chunk: for each of 4 batch-blocks, 4 matmuls per k-tile
    accumulate in 4 PSUM banks (double-buffered); the chunk partial folds
    into 16 SBUF fp32 accumulators on the vector engine (copy on first chunk,
    add after, add-into-stage + DMA out on the last chunk, whose matmuls run
    bt-major so the final drains overlap compute).
  - 9 warm-up matmuls on a zeroed scratch tile run during the DMA prologue to
    flip the PE HAM clock gate to 2.4 GHz before real data lands.
  - 157 kt x 4 bb x 4 bt = 2512 matmuls of [128x128]x[128x512] per core at
    the fp16 PE roofline (~216.8 ns each).
  - Host applies bias + tanh; output slices concatenate (no reduction).
"""

import numpy as np

P = 128
B, K, U = 2048, 20000, 4096
KPAD = 20096             # 157 * 128
KT = KPAD // P           # 157 k-tiles
USH = U // 8             # 512 units per core
BBLK = 512               # batch block
NBB = B // BBLK          # 4 batch blocks
NBT = BBLK // P          # 4 batch sub-tiles -> 4 live PSUM banks
CHUNKS = [2, 3, 4, 6, 8, 10] + [12] * 10 + [4]   # sums to 157
NWARM = 9                # warm-up matmuls (~3.8us cold: flips HAM to 2.4GHz)

TRACE = False            # set by test harness for profiled runs
LAST_RESULT = None       # BassKernelResults of the last run (for the harness)

_NC_CACHE = {}


def _build_nc():
    from concourse import bacc
    import concourse.mybir as mybir
    import concourse.tile as tile

    f32 = mybir.dt.float32
    f16 = mybir.dt.float16

    nc = bacc.Bacc("TRN2", target_bir_lowering=False, debug=False)
    xt_d = nc.dram_tensor("xt", [KPAD, B], f16, kind="ExternalInput").ap()
    w_d = nc.dram_tensor("w_sh", [KPAD, USH], f16, kind="ExternalInput").ap()
    o_d = nc.dram_tensor("out_p", [B, USH], f32, kind="ExternalOutput").ap()

    rings = [nc.sync, nc.scalar]     # the two HWDGE-capable engines
    ring_i = [0]

    def dma(dst, src):
        rings[ring_i[0] % 2].dma_start(dst, src)
        ring_i[0] += 1

    with tile.TileContext(nc) as tc:
        with (
            tc.tile_pool(name="xstream", bufs=2) as xpool,
            tc.tile_pool(name="wstream", bufs=2) as wpool,
            tc.tile_pool(name="accum", bufs=1) as apool,
            tc.tile_pool(name="stage", bufs=4) as spool,
            tc.tile_pool(name="warm", bufs=1) as warmpool,
            tc.tile_pool(name="mpsum", bufs=2, space="PSUM") as mpsum,
        ):
            acc = [
                apool.tile([P, USH], f32, tag=f"acc{i}", name=f"acc{i}")
                for i in range(NBB * NBT)
            ]
            scratch = warmpool.tile([P, BBLK], f16, tag="scr", name="scr")
            nc.vector.memset(scratch[:], 0.0)

            nchunks = len(CHUNKS)
            k0 = 0
            for c, kc in enumerate(CHUNKS):
                # chunk loads: W first (small, gates every matmul), then x in
                # per-batch-block pieces for early chunks so delivery order
                # matches consumption order.
                xc, wc = [], []
                for j in range(kc):
                    kt = k0 + j
                    wt = wpool.tile([P, USH], f16, tag=f"w{j}", name=f"w{j}")
                    dma(wt[:], w_d[kt * P:(kt + 1) * P, :])
                    wc.append(wt)
                pieces = 4 if c <= 2 else (2 if c <= 4 else 1)
                psz = B // pieces
                for j in range(kc):
                    xc.append(xpool.tile([P, B], f16, tag=f"x{j}", name=f"x{j}"))
                for q in range(pieces):
                    for j in range(kc):
                        kt = k0 + j
                        dma(
                            xc[j][:, q * psz:(q + 1) * psz],
                            xt_d[kt * P:(kt + 1) * P, q * psz:(q + 1) * psz],
                        )

                for bb in range(NBB):
                    psums = [
                        mpsum.tile([P, BBLK], f32, tag=f"ps{bt}", name=f"ps{bt}")
                        for bt in range(NBT)
                    ]
                    if c == 0 and bb == 0:
                        for wmm in range(NWARM):
                            nc.tensor.matmul(
                                psums[wmm % NBT][:],
                                scratch[:, 0:P],
                                scratch[:],
                                start=True,
                                stop=True,
                            )
                    if c == nchunks - 1:
                        # bt-major: each bank finishes early so its drain +
                        # output DMA overlaps the remaining banks' matmuls.
                        for bt in range(NBT):
                            for j in range(kc):
                                nc.tensor.matmul(
                                    psums[bt][:],
                                    xc[j][:, bb * BBLK + bt * P:bb * BBLK + (bt + 1) * P],
                                    wc[j][:],
                                    start=(j == 0),
                                    stop=(j == kc - 1),
                                )
                            st = spool.tile([P, BBLK], f32, tag="st", name="st")
                            nc.vector.tensor_add(
                                st[:], psums[bt][:], acc[bb * NBT + bt][:]
                            )
                            dma(
                                o_d[bb * BBLK + bt * P:bb * BBLK + (bt + 1) * P, :],
                                st[:],
                            )
                    else:
                        for j in range(kc):
                            for bt in range(NBT):
                                nc.tensor.matmul(
                                    psums[bt][:],
                                    xc[j][:, bb * BBLK + bt * P:bb * BBLK + (bt + 1) * P],
                                    wc[j][:],
                                    start=(j == 0),
                                    stop=(j == kc - 1),
                                )
                        for bt in range(NBT):
                            a = acc[bb * NBT + bt]
                            if c == 0:
                                nc.vector.tensor_copy(a[:], psums[bt][:])
                            else:
                                nc.vector.tensor_add(a[:], psums[bt][:], a[:])
                k0 += kc

    nc.compile()
    return nc


def _get_nc():
    if "v4" not in _NC_CACHE:
        _NC_CACHE["v4"] = _build_nc()
    return _NC_CACHE["v4"]


def kernel(x, kernel_vector, bias, nonzero_ind):
    global LAST_RESULT
    from concourse.bass_utils import run_bass_kernel_spmd

    x = np.asarray(x, dtype=np.float32)
    kernel_vector = np.asarray(kernel_vector, dtype=np.float32)
    bias = np.asarray(bias, dtype=np.float32)
    nonzero_ind = np.asarray(nonzero_ind)

    nc = _get_nc()

    # Host scatter: dense fp16 weights, K padded with zero rows to 157 tiles.
    rows = nonzero_ind[:, 0].astype(np.int64)
    cols = nonzero_ind[:, 1].astype(np.int64)
    w_acc = np.zeros(K * U, np.float32)
    np.add.at(w_acc, rows * U + cols, kernel_vector)
    w_full = np.zeros((KPAD, U), np.float16)
    w_full[:K] = w_acc.reshape(K, U)

    # Transposed, padded activations shared by all cores.
    xt_full = np.zeros((KPAD, B), np.float16)
    xt_full[:K] = x.T

    in_maps = [
        {"xt": xt_full, "w_sh": np.ascontiguousarray(w_full[:, c * USH:(c + 1) * USH])}
        for c in range(8)
    ]

    kwargs = {}
    if TRACE:
        kwargs = dict(trace=True, trace_cores=list(range(8)))
    res = run_bass_kernel_spmd(nc, in_maps, core_ids=list(range(8)), **kwargs)
    LAST_RESULT = res

    out = np.empty((B, U), np.float32)
    for c in range(8):
        out[:, c * USH:(c + 1) * USH] = res.results[c]["out_p"]
    out += bias[None, :]
    np.tanh(out, out=out)
    return out
